# revision 8
# baseline (speedup 1.0000x reference)
"""Causal multi-head attention (B=4, T=2048, D=1024, 16 heads) on 8 Trainium2
NeuronCores.

Sharding: core c = 2*b + g handles batch b (of 4) and head-group g (of 2,
8 heads each).  Each core computes Q/K/V projections for its head group,
causal attention, and a partial output projection (its 512 columns of the
out-proj contraction).  The host sums the two partial outputs per batch and
adds the bias.

v2 design (vs the phase-separated fp32r v1):
  * All matmul operands and SBUF-resident tensors are bf16 (psum stays
    fp32).  Same PE rate as fp32r but: half the DMA bytes, FWL-eligible
    weight loads, 2x DVE tensor_tensor, and half the SBUF footprint --
    which lets x stay RESIDENT (v1 streamed x from HBM twice).
  * Weight DMAs ride the ACT HWDGE ring, x/out the SP ring (parallel).
  * The exp activation table is preloaded during the initial DMA wait.
  * Work is emitted interleaved: causal attention for q-block qb overlaps
    the projections for x-tile qb+1 and the output projection for qb-1.
    The attention inner loop is ACT(exp)-bound (~1us/iter vs ~0.65us of
    PE work), so the interleaved projection matmuls fill PE into the
    exp-wait bubbles, and conversely attention's exp fills the ACT engine
    while projections own PE.
  * Normalization multiplies straight out of ctx PSUM (one fewer DVE
    copy per head than v1).

On-core layout:
  xsb    [128, 8, 2048]   resident x^T (dc-chunk, q)        bf16
  QT,KT  [128, 4, 2048]   (dg within head-pair, pair, q)    bf16
  V      [128, 16, 8, 65] (k within chunk, k-chunk, head, dv | ones) bf16
  ctxT   [128, 4, 2048]   (dv within pair, pair, q)         bf16
Attention per (q-block 512, head-pair): transposed scores ST[k, 2, q] via two
row-tiled K=64 matmuls (base partitions 0/64), exp(S/8) on ACT (no max
subtraction: |S|/8 <= ~3), causal triangle as post-exp 0/1 bf16 multiply,
PV matmul with lhsT=[V_h|ones] (M=65) accumulating ctx and the softmax
denominator, then reciprocal (DVE) + ones-matmul partition broadcast (PE)
+ DVE multiply (psum x sbuf) into bf16 ctxT.
"""
from contextlib import ExitStack, nullcontext

import numpy as np

import concourse.bass as bass
import concourse.mybir as mybir
import concourse.tile as tile
from concourse import bacc
from concourse.bass_utils import run_bass_kernel_spmd

B, T, D = 4, 2048, 1024
NH, HDIM = 16, 64
GH = 8           # heads per core
DG = 512         # head dims per core
P = 128
NPAIR = 4        # head pairs per core
QB = 512         # q block width
NQB = T // QB
NKC = T // P     # k chunks of 128
NDC = D // P     # d chunks of 128
SCALE = 1.0 / np.sqrt(HDIM)

BF16 = mybir.dt.bfloat16
F32 = mybir.dt.float32
F32R = mybir.dt.float32r
AF = mybir.ActivationFunctionType

_CACHE = {}
PT_BUFS = 3
ST_BUFS = 2
INTERLEAVE = True
GP_BCAST = True  # partition-broadcast of 1/denom on GpSimd instead of PE


def _build(loop_n=None, interleave=None):
    if interleave is None:
        interleave = INTERLEAVE
    nc = bacc.Bacc("TRN2", target_bir_lowering=False, debug=False, num_devices=8)
    xT = nc.dram_tensor("xt", [D, T], BF16, kind="ExternalInput").ap()
    wq = nc.dram_tensor("wq", [D, DG], BF16, kind="ExternalInput").ap()
    wk = nc.dram_tensor("wk", [D, DG], BF16, kind="ExternalInput").ap()
    wv = nc.dram_tensor("wv", [D, DG], BF16, kind="ExternalInput").ap()
    wo = nc.dram_tensor("wo", [DG, D], BF16, kind="ExternalInput").ap()
    tri = nc.dram_tensor("tri", [P, P], BF16, kind="ExternalInput").ap()
    ones = nc.dram_tensor("ones", [P, P], BF16, kind="ExternalInput").ap()
    out = nc.dram_tensor("out", [T, D], F32, kind="ExternalOutput").ap()

    xT_r = xT.rearrange("(dc p) q -> p dc q", p=P)
    wq_r = wq.rearrange("(dc p) n -> p dc n", p=P)
    wk_r = wk.rearrange("(dc p) n -> p dc n", p=P)
    wv_r = wv.rearrange("(dc p) n -> p dc n", p=P)
    wo_r = wo.rearrange("(c p) n -> p c n", p=P)
    out_r = out.rearrange("(qc p) n -> qc p n", p=P)

    with tile.TileContext(nc) as tc:
        with ExitStack() as top:
            pers = top.enter_context(tc.tile_pool(name="persist", bufs=1))
            xsb = pers.tile([P, NDC, T], BF16)
            qt_sb = pers.tile([P, NPAIR, T], BF16)
            kt_sb = pers.tile([P, NPAIR, T], BF16)
            v_sb = pers.tile([P, NKC, GH, HDIM + 1], BF16)
            ctxT = pers.tile([P, NPAIR, T], BF16)
            wq_sb = pers.tile([P, NDC, DG], BF16)
            wk_sb = pers.tile([P, NDC, DG], BF16)
            wv_sb = pers.tile([P, NDC, DG], BF16)
            wo_sb = pers.tile([P, NPAIR, D], BF16)
            tri_sb = pers.tile([P, P], BF16)
            ones_sb = pers.tile([P, P], BF16)
            warm = pers.tile([1, 2], BF16)

            # x + small consts on the SP HWDGE ring; weights on the ACT ring
            for xi in range(NQB):
                nc.sync.dma_start(
                    xsb[:, :, xi * QB:(xi + 1) * QB],
                    xT_r[:, :, xi * QB:(xi + 1) * QB],
                )
            nc.sync.dma_start(tri_sb[:], tri)
            nc.sync.dma_start(ones_sb[:], ones)
            nc.scalar.dma_start(wq_sb[:], wq_r)
            nc.scalar.dma_start(wk_sb[:], wk_r)
            nc.scalar.dma_start(wv_sb[:], wv_r)
            nc.scalar.dma_start(wo_sb[:], wo_r)
            # preload the exp table while DMAs stream
            with nc.allow_low_precision(reason="table warmup"):
                nc.scalar.activation(warm[0:1, :], ones_sb[0:1, 0:2], AF.Exp)
            # ones-column of V (denominator trick)
            with nc.allow_low_precision(reason="bf16 store"):
                nc.vector.tensor_copy(
                    v_sb[:, :, :, HDIM],
                    ones_sb.rearrange("p (a b) -> p a b", a=NKC, b=GH),
                )

            body = ExitStack()
            st_psp = body.enter_context(
                tc.tile_pool(name="st_ps", bufs=ST_BUFS, space="PSUM"))
            ctx_psp = body.enter_context(
                tc.tile_pool(name="ctx_ps", bufs=2, space="PSUM"))
            genp = body.enter_context(
                tc.tile_pool(name="gen_ps", bufs=2, space="PSUM"))
            ptp = body.enter_context(tc.tile_pool(name="ptp", bufs=PT_BUFS))
            rcp = body.enter_context(tc.tile_pool(name="rcp", bufs=2))
            bcsp = body.enter_context(tc.tile_pool(name="bcsp", bufs=2))
            cup = body.enter_context(tc.tile_pool(name="cup", bufs=4))
            otp = body.enter_context(tc.tile_pool(name="otp", bufs=3))

            # ---- work units ----
            def qk_unit(w_sb, dst, pair, xi):
                qcols = slice(xi * QB, (xi + 1) * QB)
                pps = genp.tile([P, QB], F32, name="gen", tag="gen")
                for dc in range(NDC):
                    nc.tensor.matmul(
                        pps[:],
                        w_sb[:, dc, pair * P:(pair + 1) * P],
                        xsb[:, dc, qcols],
                        start=(dc == 0), stop=(dc == NDC - 1),
                    )
                with nc.allow_low_precision(reason="bf16 store"):
                    nc.vector.tensor_copy(dst[:, pair, qcols], pps[:])

            def v_unit(kc):
                vps = genp.tile([P, DG], F32, name="gen", tag="gen")
                for dc in range(NDC):
                    nc.tensor.matmul(
                        vps[:],
                        xsb[:, dc, kc * P:(kc + 1) * P],
                        wv_sb[:, dc, :],
                        start=(dc == 0), stop=(dc == NDC - 1),
                    )
                with nc.allow_low_precision(reason="bf16 store"):
                    nc.vector.tensor_copy(
                        v_sb[:, kc, :, 0:HDIM],
                        vps.rearrange("p (h d) -> p h d", d=HDIM),
                    )

            def att_iter(qb, pair, kc, nkc, ctxp):
                r = P * kc - QB * qb
                lo = max(r, 0)
                st = st_psp.tile([P, 2, QB], F32, name="stps", tag="st")
                pt = ptp.tile([P, 2, QB], BF16, name="pt")
                for hi in range(2):
                    nc.tensor.matmul(
                        st[:, hi, lo:QB],
                        kt_sb[HDIM * hi:HDIM * (hi + 1), pair,
                              kc * P:(kc + 1) * P],
                        qt_sb[HDIM * hi:HDIM * (hi + 1), pair,
                              qb * QB + lo:(qb + 1) * QB],
                        start=True, stop=True,
                    )
                with nc.allow_low_precision(reason="bf16 probs"):
                    nc.scalar.activation(
                        pt[:, :, lo:QB], st[:, :, lo:QB], AF.Exp,
                        scale=float(SCALE))
                if r >= 0:
                    for hi in range(2):
                        with nc.allow_low_precision(reason="bf16 probs"):
                            nc.vector.tensor_tensor(
                                pt[:, hi, r:r + P],
                                pt[:, hi, r:r + P],
                                tri_sb[:],
                                mybir.AluOpType.mult,
                            )
                for hi in range(2):
                    nc.tensor.matmul(
                        ctxp[hi][:, lo:QB],
                        v_sb[:, kc, 2 * pair + hi, :],
                        pt[:, hi, lo:QB],
                        start=(kc == 0), stop=(kc == nkc - 1),
                    )

            def norm(qb, pair, ctxp):
                # evacuate ctx+denom to SBUF first: releases PSUM fast and
                # takes the normalize chain off the PV critical path
                qcols = slice(qb * QB, (qb + 1) * QB)
                for hi in range(2):
                    ctxu = cup.tile([HDIM + 1, QB], BF16, name="ctxu")
                    with nc.allow_low_precision(reason="bf16 ctx"):
                        nc.vector.tensor_copy(ctxu[:], ctxp[hi][:])
                    recip = rcp.tile([1, QB], F32R, name="recip")
                    with nc.allow_low_precision(reason="fp32r recip"):
                        nc.vector.reciprocal(recip[0:1, :],
                                             ctxu[HDIM:HDIM + 1, :])
                    bcs = bcsp.tile([HDIM, QB], F32R, name="bcs")
                    if GP_BCAST:
                        nc.gpsimd.partition_broadcast(bcs[:], recip[0:1, :])
                    else:
                        bc = genp.tile([P, QB], F32, name="gen", tag="gen")
                        nc.tensor.matmul(
                            bc[0:HDIM, :], ones_sb[0:1, 0:HDIM],
                            recip[0:1, :], start=True, stop=True)
                        with nc.allow_low_precision(reason="fp32r scale"):
                            nc.vector.tensor_copy(bcs[:], bc[0:HDIM, :])
                    with nc.allow_low_precision(reason="bf16 ctx"):
                        nc.vector.tensor_tensor(
                            ctxT[HDIM * hi:HDIM * (hi + 1), pair, qcols],
                            ctxu[0:HDIM, :],
                            bcs[:],
                            mybir.AluOpType.mult,
                        )

            def out_unit(qc):
                ot = otp.tile([P, D], F32, name="ot")
                for ob in range(2):
                    ops = genp.tile([P, 512], F32, name="gen", tag="gen")
                    for c in range(NPAIR):
                        nc.tensor.matmul(
                            ops[:],
                            ctxT[:, c, qc * P:(qc + 1) * P],
                            wo_sb[:, c, ob * 512:(ob + 1) * 512],
                            start=(c == 0), stop=(c == NPAIR - 1),
                        )
                    nc.vector.tensor_copy(ot[:, ob * 512:(ob + 1) * 512],
                                          ops[:])
                nc.sync.dma_start(out_r[qc], ot[:])

            def proj_round(xi):
                units = []
                for w_sb, dst in ((wq_sb, qt_sb), (wk_sb, kt_sb)):
                    for pair in range(NPAIR):
                        units.append(lambda w=w_sb, d=dst, p=pair, x=xi:
                                     qk_unit(w, d, p, x))
                for kl in range(QB // P):
                    units.append(lambda k=xi * (QB // P) + kl: v_unit(k))
                return units

            def out_round(qb):
                return [lambda q=4 * qb + j: out_unit(q) for j in range(4)]

            # ---- emission ----
            lp = (tc.For_i(0, loop_n, 1, hint_engines=(mybir.EngineType.PE,))
                  if loop_n else nullcontext())
            with lp:
                if interleave:
                    for u in proj_round(0):
                        u()
                    for qb in range(NQB):
                        fillers = []
                        if qb + 1 < NQB:
                            fillers += proj_round(qb + 1)
                        if qb - 1 >= 0:
                            fillers += out_round(qb - 1)
                        nkc = (QB // P) * (qb + 1)
                        n_att = NPAIR * nkc
                        # spread fillers evenly among attention iterations
                        fi = 0
                        ai = 0
                        for pair in range(NPAIR):
                            ctxp = [
                                ctx_psp.tile([HDIM + 1, QB], F32,
                                             name="ctxps", tag="ctx")
                                for _ in range(2)
                            ]
                            for kc in range(nkc):
                                att_iter(qb, pair, kc, nkc, ctxp)
                                ai += 1
                                while fi * n_att < ai * len(fillers):
                                    fillers[fi]()
                                    fi += 1
                            norm(qb, pair, ctxp)
                        while fi < len(fillers):
                            fillers[fi]()
                            fi += 1
                    for u in out_round(NQB - 1):
                        u()
                else:
                    for xi in range(NQB):
                        for u in proj_round(xi):
                            u()
                    for qb in range(NQB):
                        nkc = (QB // P) * (qb + 1)
                        for pair in range(NPAIR):
                            ctxp = [
                                ctx_psp.tile([HDIM + 1, QB], F32,
                                             name="ctxps", tag="ctx")
                                for _ in range(2)
                            ]
                            for kc in range(nkc):
                                att_iter(qb, pair, kc, nkc, ctxp)
                            norm(qb, pair, ctxp)
                    for qb in range(NQB):
                        for u in out_round(qb):
                            u()
            body.close()

    nc.compile()
    return nc


def _get_nc():
    if "nc" not in _CACHE:
        _CACHE["nc"] = _build()
    return _CACHE["nc"]


def make_in_maps(inputs, W_q, W_k, W_v, W_o):
    bf16 = mybir.dt.np(BF16)
    x = np.asarray(inputs, dtype=np.float32)
    W_q = np.asarray(W_q, dtype=np.float32)
    W_k = np.asarray(W_k, dtype=np.float32)
    W_v = np.asarray(W_v, dtype=np.float32)
    W_o = np.asarray(W_o, dtype=np.float32)
    tri = np.where(
        np.arange(P)[:, None] <= np.arange(P)[None, :], 1.0, 0.0
    ).astype(bf16)
    ones = np.ones((P, P), dtype=bf16)
    in_maps = []
    for c in range(8):
        b, g = divmod(c, 2)
        gs = slice(g * DG, (g + 1) * DG)
        in_maps.append({
            "xt": np.ascontiguousarray(x[b].T).astype(bf16),
            "wq": np.ascontiguousarray(W_q[gs, :].T).astype(bf16),
            "wk": np.ascontiguousarray(W_k[gs, :].T).astype(bf16),
            "wv": np.ascontiguousarray(W_v[gs, :].T).astype(bf16),
            "wo": np.ascontiguousarray(W_o[:, gs].T).astype(bf16),
            "tri": tri,
            "ones": ones,
        })
    return in_maps


def combine(results, b_o):
    b_o = np.asarray(b_o, dtype=np.float32)
    out = np.empty((B, T, D), dtype=np.float32)
    for b in range(B):
        out[b] = results[2 * b]["out"] + results[2 * b + 1]["out"] + b_o
    return out


def kernel(inputs, W_q, W_k, W_v, W_o, b_o):
    nc = _get_nc()
    in_maps = make_in_maps(inputs, W_q, W_k, W_v, W_o)
    res = run_bass_kernel_spmd(nc, in_maps, core_ids=list(range(8)), trace=False)
    return combine(res.results, b_o)


# revision 37
# speedup vs baseline: 1.1328x; 1.1328x over previous
"""Causal multi-head attention (B=4, T=2048, D=1024, 16 heads) on 8 Trainium2
NeuronCores.

Sharding: core c = 2*b + g handles batch b (of 4) and head-group g (of 2,
8 heads each).  Each core computes Q/K/V projections for its head group,
causal attention, and a partial output projection (its 512 columns of the
out-proj contraction).  The host sums the two partial outputs per batch and
adds the bias.

v2 design (vs the phase-separated fp32r v1):
  * All matmul operands and SBUF-resident tensors are bf16 (psum stays
    fp32).  Same PE rate as fp32r but: half the DMA bytes, FWL-eligible
    weight loads, 2x DVE tensor_tensor, and half the SBUF footprint --
    which lets x stay RESIDENT (v1 streamed x from HBM twice).
  * Weight DMAs ride the ACT HWDGE ring, x/out the SP ring (parallel).
  * The exp activation table is preloaded during the initial DMA wait.
  * Work is emitted interleaved: causal attention for q-block qb overlaps
    the projections for x-tile qb+1 and the output projection for qb-1.
    The attention inner loop is ACT(exp)-bound (~1us/iter vs ~0.65us of
    PE work), so the interleaved projection matmuls fill PE into the
    exp-wait bubbles, and conversely attention's exp fills the ACT engine
    while projections own PE.
  * Normalization multiplies straight out of ctx PSUM (one fewer DVE
    copy per head than v1).

On-core layout:
  xsb    [128, 8, 2048]   resident x^T (dc-chunk, q)        bf16
  QT,KT  [128, 4, 2048]   (dg within head-pair, pair, q)    bf16
  V      [128, 16, 8, 65] (k within chunk, k-chunk, head, dv | ones) bf16
  ctxT   [128, 4, 2048]   (dv within pair, pair, q)         bf16
Attention per (q-block 512, head-pair): transposed scores ST[k, 2, q] via two
row-tiled K=64 matmuls (base partitions 0/64), exp(S/8) on ACT (no max
subtraction: |S|/8 <= ~3), causal triangle as post-exp 0/1 bf16 multiply,
PV matmul with lhsT=[V_h|ones] (M=65) accumulating ctx and the softmax
denominator, then reciprocal (DVE) + ones-matmul partition broadcast (PE)
+ DVE multiply (psum x sbuf) into bf16 ctxT.
"""
from contextlib import ExitStack, nullcontext

import numpy as np

import concourse.bass as bass
import concourse.mybir as mybir
import concourse.tile as tile
from concourse import bacc
from concourse.bass_utils import run_bass_kernel_spmd

B, T, D = 4, 2048, 1024
NH, HDIM = 16, 64
GH = 8           # heads per core
DG = 512         # head dims per core
P = 128
NPAIR = 4        # head pairs per core
QB = 512         # q block width
NQB = T // QB
NKC = T // P     # k chunks of 128
NDC = D // P     # d chunks of 128
SCALE = 1.0 / np.sqrt(HDIM)

BF16 = mybir.dt.bfloat16
F32 = mybir.dt.float32
F32R = mybir.dt.float32r
AF = mybir.ActivationFunctionType

_CACHE = {}
PT_BUFS = 3
ST_BUFS = 2
INTERLEAVE = True


def _build(loop_n=None, interleave=None, tiny_exp=False, no_recip=False,
           no_mask=False):
    """tiny_exp/no_recip/no_mask are timing-only diagnostics that cripple
    correctness."""
    if interleave is None:
        interleave = INTERLEAVE
    nc = bacc.Bacc("TRN2", target_bir_lowering=False, debug=False, num_devices=8)
    xT = nc.dram_tensor("xt", [D, T], BF16, kind="ExternalInput").ap()
    wq = nc.dram_tensor("wq", [D, DG], BF16, kind="ExternalInput").ap()
    wk = nc.dram_tensor("wk", [D, DG], BF16, kind="ExternalInput").ap()
    wv = nc.dram_tensor("wv", [D, DG], BF16, kind="ExternalInput").ap()
    wo = nc.dram_tensor("wo", [DG, D], BF16, kind="ExternalInput").ap()
    tri = nc.dram_tensor("tri", [P, P], BF16, kind="ExternalInput").ap()
    ones = nc.dram_tensor("ones", [P, P], BF16, kind="ExternalInput").ap()
    # esel[:, 128p:128(p+1)] selects/broadcasts recip-tile row 32*((2p+hi)%4)
    # into output partition half hi (norm's selector/broadcast matmul)
    esel = nc.dram_tensor("esel", [P, NPAIR * P], F32R,
                          kind="ExternalInput").ap()
    out = nc.dram_tensor("out", [T, D], F32, kind="ExternalOutput").ap()

    xT_r = xT.rearrange("(dc p) q -> p dc q", p=P)
    wq_r = wq.rearrange("(dc p) n -> p dc n", p=P)
    wk_r = wk.rearrange("(dc p) n -> p dc n", p=P)
    wv_r = wv.rearrange("(dc p) n -> p dc n", p=P)
    wo_r = wo.rearrange("(c p) n -> p c n", p=P)
    out_r = out.rearrange("(qc p) n -> qc p n", p=P)

    with tile.TileContext(nc) as tc:
        with ExitStack() as top:
            pers = top.enter_context(tc.tile_pool(name="persist", bufs=1))
            xsb = pers.tile([P, NDC, T], BF16)
            qt_sb = pers.tile([P, NPAIR, T], BF16)
            kt_sb = pers.tile([P, NPAIR, T], BF16)
            v_sb = pers.tile([P, NKC, GH, HDIM + 1], BF16)
            ctxT = pers.tile([P, NPAIR, T], BF16)
            wq_sb = pers.tile([P, NDC, DG], BF16)
            wk_sb = pers.tile([P, NDC, DG], BF16)
            wv_sb = pers.tile([P, NDC, DG], BF16)
            wo_sb = pers.tile([P, NPAIR, D], BF16)
            tri_sb = pers.tile([P, P], BF16)
            ones_sb = pers.tile([P, P], BF16)
            esel_sb = pers.tile([P, NPAIR * P], F32R)
            warm = pers.tile([1, 2], BF16)

            # x + small consts on the SP HWDGE ring; weights on the ACT ring
            for xi in range(NQB):
                nc.sync.dma_start(
                    xsb[:, :, xi * QB:(xi + 1) * QB],
                    xT_r[:, :, xi * QB:(xi + 1) * QB],
                )
            nc.sync.dma_start(tri_sb[:], tri)
            nc.sync.dma_start(ones_sb[:], ones)
            nc.sync.dma_start(esel_sb[:], esel)
            nc.scalar.dma_start(wq_sb[:], wq_r)
            nc.scalar.dma_start(wk_sb[:], wk_r)
            nc.scalar.dma_start(wv_sb[:], wv_r)
            nc.scalar.dma_start(wo_sb[:], wo_r)
            # preload the exp table while DMAs stream
            with nc.allow_low_precision(reason="table warmup"):
                nc.scalar.activation(warm[0:1, :], ones_sb[0:1, 0:2], AF.Exp)
            # ones-column of V (denominator trick)
            with nc.allow_low_precision(reason="bf16 store"):
                nc.vector.tensor_copy(
                    v_sb[:, :, :, HDIM],
                    ones_sb.rearrange("p (a b) -> p a b", a=NKC, b=GH),
                )

            body = ExitStack()
            st_psp = body.enter_context(
                tc.tile_pool(name="st_ps", bufs=ST_BUFS, space="PSUM"))
            ctx_psp = body.enter_context(
                tc.tile_pool(name="ctx_ps", bufs=2, space="PSUM"))
            genp = body.enter_context(
                tc.tile_pool(name="gen_ps", bufs=2, space="PSUM"))
            ptp = body.enter_context(tc.tile_pool(name="ptp", bufs=PT_BUFS))
            rcp = body.enter_context(tc.tile_pool(name="rcp", bufs=2))
            cup = body.enter_context(tc.tile_pool(name="cup", bufs=10))
            otp = body.enter_context(tc.tile_pool(name="otp", bufs=3))

            # ---- work units ----
            def qk_unit(w_sb, dst, pair, xi):
                qcols = slice(xi * QB, (xi + 1) * QB)
                pps = genp.tile([P, QB], F32, name="gen", tag="gen")
                for dc in range(NDC):
                    nc.tensor.matmul(
                        pps[:],
                        w_sb[:, dc, pair * P:(pair + 1) * P],
                        xsb[:, dc, qcols],
                        start=(dc == 0), stop=(dc == NDC - 1),
                    )
                with nc.allow_low_precision(reason="bf16 store"):
                    nc.vector.tensor_copy(dst[:, pair, qcols], pps[:])

            def v_unit(kc):
                vps = genp.tile([P, DG], F32, name="gen", tag="gen")
                for dc in range(NDC):
                    nc.tensor.matmul(
                        vps[:],
                        xsb[:, dc, kc * P:(kc + 1) * P],
                        wv_sb[:, dc, :],
                        start=(dc == 0), stop=(dc == NDC - 1),
                    )
                with nc.allow_low_precision(reason="bf16 store"):
                    nc.vector.tensor_copy(
                        v_sb[:, kc, :, 0:HDIM],
                        vps.rearrange("p (h d) -> p h d", d=HDIM),
                    )

            def att_iter(qb, pair, kc, nkc, ctxp):
                r = P * kc - QB * qb
                lo = max(r, 0)
                st = st_psp.tile([P, 2, QB], F32, name="stps", tag="st")
                pt = ptp.tile([P, 2, QB], BF16, name="pt")
                for hi in range(2):
                    nc.tensor.matmul(
                        st[:, hi, lo:QB],
                        kt_sb[HDIM * hi:HDIM * (hi + 1), pair,
                              kc * P:(kc + 1) * P],
                        qt_sb[HDIM * hi:HDIM * (hi + 1), pair,
                              qb * QB + lo:(qb + 1) * QB],
                        start=True, stop=True,
                    )
                ehi = lo + 2 if tiny_exp else QB
                with nc.allow_low_precision(reason="bf16 probs"):
                    nc.scalar.activation(
                        pt[:, :, lo:ehi], st[:, :, lo:ehi], AF.Exp,
                        scale=float(SCALE))
                if r >= 0 and not no_mask:
                    for hi in range(2):
                        with nc.allow_low_precision(reason="bf16 probs"):
                            nc.vector.tensor_tensor(
                                pt[:, hi, r:r + P],
                                pt[:, hi, r:r + P],
                                tri_sb[:],
                                mybir.AluOpType.mult,
                            )
                for hi in range(2):
                    nc.tensor.matmul(
                        ctxp[hi][:, lo:QB],
                        v_sb[:, kc, 2 * pair + hi, :],
                        pt[:, hi, lo:QB],
                        start=(kc == 0), stop=(kc == nkc - 1),
                    )

            def evac(qb, pair, ctxp, dens, ctxus):
                # evacuate ctx to SBUF (releases PSUM fast) and gather the
                # denominator row into a den tile for the per-qb batched
                # reciprocal.  Engine APs may only start at partition
                # 0/32/64/96, so j=2*pair+hi maps to tile j//4, row 32*(j%4).
                for hi in range(2):
                    ctxu = cup.tile([HDIM, QB], BF16, name="ctxu")
                    with nc.allow_low_precision(reason="bf16 ctx"):
                        nc.vector.tensor_copy(ctxu[:], ctxp[hi][0:HDIM, :])
                    j = 2 * pair + hi
                    ctxus[j] = ctxu
                    r = 32 * (j % 4)
                    nc.vector.tensor_copy(
                        dens[j // 4][r:r + 1, :],
                        ctxp[hi][HDIM:HDIM + 1, :])

            def norm(qb, dens, ctxus):
                # batched reciprocal: DVE reciprocal is iterative
                # (~8 cyc/elem on HW) and free-size-bound, so one call per
                # den tile (4 denominators on partitions 0/32/64/96) is ~4x
                # cheaper than per-head [1, QB] calls.  Unused partitions
                # hold garbage whose reciprocal is never read.  Then one
                # K=128 selector matmul per pair broadcasts both heads'
                # recip rows into [128, QB] PSUM and the normalize multiply
                # reads it straight from there.
                qcols = slice(qb * QB, (qb + 1) * QB)
                recips = []
                for t in range(2):
                    rc = rcp.tile([P, QB], F32R, name=f"recip{t}",
                                  tag=f"recip{t}")
                    with nc.allow_low_precision(reason="fp32r recip"):
                        if no_recip:
                            nc.vector.tensor_copy(rc[:], dens[t][:])
                        else:
                            nc.vector.reciprocal(rc[:], dens[t][:])
                    recips.append(rc)
                for pair in range(NPAIR):
                    bc = genp.tile([P, QB], F32, name="gen", tag="gen")
                    nc.tensor.matmul(
                        bc[:], esel_sb[:, P * pair:P * (pair + 1)],
                        recips[pair // 2][:, :], start=True, stop=True)
                    for hi in range(2):
                        j = 2 * pair + hi
                        with nc.allow_low_precision(reason="bf16 ctx"):
                            nc.vector.tensor_tensor(
                                ctxT[HDIM * hi:HDIM * (hi + 1), pair, qcols],
                                ctxus[j][:],
                                bc[HDIM * hi:HDIM * (hi + 1), :],
                                mybir.AluOpType.mult,
                            )

            def out_unit(qc):
                ot = otp.tile([P, D], F32, name="ot")
                for ob in range(2):
                    ops = genp.tile([P, 512], F32, name="gen", tag="gen")
                    for c in range(NPAIR):
                        nc.tensor.matmul(
                            ops[:],
                            ctxT[:, c, qc * P:(qc + 1) * P],
                            wo_sb[:, c, ob * 512:(ob + 1) * 512],
                            start=(c == 0), stop=(c == NPAIR - 1),
                        )
                    nc.vector.tensor_copy(ot[:, ob * 512:(ob + 1) * 512],
                                          ops[:])
                nc.sync.dma_start(out_r[qc], ot[:])

            def proj_round(xi):
                units = []
                for w_sb, dst in ((wq_sb, qt_sb), (wk_sb, kt_sb)):
                    for pair in range(NPAIR):
                        units.append(lambda w=w_sb, d=dst, p=pair, x=xi:
                                     qk_unit(w, d, p, x))
                for kl in range(QB // P):
                    units.append(lambda k=xi * (QB // P) + kl: v_unit(k))
                return units

            def out_round(qb):
                return [lambda q=4 * qb + j: out_unit(q) for j in range(4)]

            # ---- emission ----
            lp = (tc.For_i(0, loop_n, 1, hint_engines=(mybir.EngineType.PE,))
                  if loop_n else nullcontext())
            with lp:
                if interleave:
                    for u in proj_round(0):
                        u()
                    for qb in range(NQB):
                        fillers = []
                        if qb + 1 < NQB:
                            fillers += proj_round(qb + 1)
                        if qb - 1 >= 0:
                            fillers += out_round(qb - 1)
                        nkc = (QB // P) * (qb + 1)
                        n_att = NPAIR * nkc
                        dens = [rcp.tile([P, QB], F32, name=f"den{t}",
                                         tag=f"den{t}") for t in range(2)]
                        for d in dens:
                            # unused partitions must be finite: the selector
                            # matmul reads all 128 rows (0 * inf = NaN)
                            nc.gpsimd.memset(d[:], 1.0)
                        ctxus = [None] * (2 * NPAIR)
                        # spread fillers evenly among attention iterations
                        fi = 0
                        ai = 0
                        for pair in range(NPAIR):
                            ctxp = [
                                ctx_psp.tile([HDIM + 1, QB], F32,
                                             name="ctxps", tag="ctx")
                                for _ in range(2)
                            ]
                            for kc in range(nkc):
                                att_iter(qb, pair, kc, nkc, ctxp)
                                ai += 1
                                while fi * n_att < ai * len(fillers):
                                    fillers[fi]()
                                    fi += 1
                            evac(qb, pair, ctxp, dens, ctxus)
                        while fi < len(fillers):
                            fillers[fi]()
                            fi += 1
                        norm(qb, dens, ctxus)
                    for u in out_round(NQB - 1):
                        u()
                else:
                    for xi in range(NQB):
                        for u in proj_round(xi):
                            u()
                    for qb in range(NQB):
                        nkc = (QB // P) * (qb + 1)
                        dens = [rcp.tile([P, QB], F32, name=f"den{t}",
                                         tag=f"den{t}") for t in range(2)]
                        for d in dens:
                            # unused partitions must be finite: the selector
                            # matmul reads all 128 rows (0 * inf = NaN)
                            nc.gpsimd.memset(d[:], 1.0)
                        ctxus = [None] * (2 * NPAIR)
                        for pair in range(NPAIR):
                            ctxp = [
                                ctx_psp.tile([HDIM + 1, QB], F32,
                                             name="ctxps", tag="ctx")
                                for _ in range(2)
                            ]
                            for kc in range(nkc):
                                att_iter(qb, pair, kc, nkc, ctxp)
                            evac(qb, pair, ctxp, dens, ctxus)
                        norm(qb, dens, ctxus)
                    for qb in range(NQB):
                        for u in out_round(qb):
                            u()
            body.close()

    nc.compile()
    return nc


def _get_nc():
    if "nc" not in _CACHE:
        _CACHE["nc"] = _build()
    return _CACHE["nc"]


def make_in_maps(inputs, W_q, W_k, W_v, W_o):
    bf16 = mybir.dt.np(BF16)
    x = np.asarray(inputs, dtype=np.float32)
    W_q = np.asarray(W_q, dtype=np.float32)
    W_k = np.asarray(W_k, dtype=np.float32)
    W_v = np.asarray(W_v, dtype=np.float32)
    W_o = np.asarray(W_o, dtype=np.float32)
    tri = np.where(
        np.arange(P)[:, None] <= np.arange(P)[None, :], 1.0, 0.0
    ).astype(bf16)
    ones = np.ones((P, P), dtype=bf16)
    k_idx = np.arange(P)[:, None]
    m_idx = np.arange(NPAIR * P)[None, :]
    esel = (k_idx == 32 * ((2 * (m_idx // P) + (m_idx % P) // HDIM) % 4)
            ).astype(np.float32)
    in_maps = []
    for c in range(8):
        b, g = divmod(c, 2)
        gs = slice(g * DG, (g + 1) * DG)
        in_maps.append({
            "xt": np.ascontiguousarray(x[b].T).astype(bf16),
            "wq": np.ascontiguousarray(W_q[gs, :].T).astype(bf16),
            "wk": np.ascontiguousarray(W_k[gs, :].T).astype(bf16),
            "wv": np.ascontiguousarray(W_v[gs, :].T).astype(bf16),
            "wo": np.ascontiguousarray(W_o[:, gs].T).astype(bf16),
            "tri": tri,
            "ones": ones,
            "esel": esel,
        })
    return in_maps


def combine(results, b_o):
    b_o = np.asarray(b_o, dtype=np.float32)
    out = np.empty((B, T, D), dtype=np.float32)
    for b in range(B):
        out[b] = results[2 * b]["out"] + results[2 * b + 1]["out"] + b_o
    return out


def kernel(inputs, W_q, W_k, W_v, W_o, b_o):
    nc = _get_nc()
    in_maps = make_in_maps(inputs, W_q, W_k, W_v, W_o)
    res = run_bass_kernel_spmd(nc, in_maps, core_ids=list(range(8)), trace=False)
    return combine(res.results, b_o)


# revision 41
# speedup vs baseline: 1.1450x; 1.0108x over previous
"""Causal multi-head attention (B=4, T=2048, D=1024, 16 heads) on 8 Trainium2
NeuronCores.

Sharding: core c = 2*b + g handles batch b (of 4) and head-group g (of 2,
8 heads each).  Each core computes Q/K/V projections for its head group,
causal attention, and a partial output projection (its 512 columns of the
out-proj contraction).  The host sums the two partial outputs per batch and
adds the bias.

v2 design (vs the phase-separated fp32r v1):
  * All matmul operands and SBUF-resident tensors are bf16 (psum stays
    fp32).  Same PE rate as fp32r but: half the DMA bytes, FWL-eligible
    weight loads, 2x DVE tensor_tensor, and half the SBUF footprint --
    which lets x stay RESIDENT (v1 streamed x from HBM twice).
  * Weight DMAs ride the ACT HWDGE ring, x/out the SP ring (parallel).
  * The exp activation table is preloaded during the initial DMA wait.
  * Work is emitted interleaved: causal attention for q-block qb overlaps
    the projections for x-tile qb+1 and the output projection for qb-1.
    The attention inner loop is ACT(exp)-bound (~1us/iter vs ~0.65us of
    PE work), so the interleaved projection matmuls fill PE into the
    exp-wait bubbles, and conversely attention's exp fills the ACT engine
    while projections own PE.
  * Softmax denominators are gathered (4 per tile, on the legal engine
    partition bases 0/32/64/96) and inverted in two batched DVE
    reciprocals per q-block: HW reciprocal is iterative (~8 cyc/elem)
    and free-size-bound, so 8 per-head [1, QB] calls cost ~8x more than
    2 batched ones (measured -44us wall).  A K=128 selector matmul
    (esel) then broadcasts both heads' recip rows into [128, QB] PSUM
    and the normalize multiply reads it straight from there.

On-core layout:
  xsb    [128, 8, 2048]   resident x^T (dc-chunk, q)        bf16
  QT,KT  [128, 4, 2048]   (dg within head-pair, pair, q)    bf16
  V      [128, 16, 8, 65] (k within chunk, k-chunk, head, dv | ones) bf16
  ctxT   [128, 4, 2048]   (dv within pair, pair, q)         bf16
Attention per (q-block 512, head-pair): transposed scores ST[k, 2, q] via two
row-tiled K=64 matmuls (base partitions 0/64), exp(S/8) on ACT (no max
subtraction: |S|/8 <= ~3), causal triangle as post-exp 0/1 bf16 multiply
on DVE (gpsimd measured worse), PV matmul with lhsT=[V_h|ones] (M=65)
accumulating ctx and the softmax denominator.

Measured (pairwise loop-delta, median): 487us (v1 fp32r) -> 368us (bf16 +
interleave + resident x) -> 325us (batched reciprocal).
"""
from contextlib import ExitStack, nullcontext

import numpy as np

import concourse.bass as bass
import concourse.mybir as mybir
import concourse.tile as tile
from concourse import bacc
from concourse.bass_utils import run_bass_kernel_spmd

B, T, D = 4, 2048, 1024
NH, HDIM = 16, 64
GH = 8           # heads per core
DG = 512         # head dims per core
P = 128
NPAIR = 4        # head pairs per core
QB = 512         # q block width
NQB = T // QB
NKC = T // P     # k chunks of 128
NDC = D // P     # d chunks of 128
SCALE = 1.0 / np.sqrt(HDIM)

BF16 = mybir.dt.bfloat16
F32 = mybir.dt.float32
F32R = mybir.dt.float32r
AF = mybir.ActivationFunctionType

_CACHE = {}
PT_BUFS = 3
ST_BUFS = 2
INTERLEAVE = True


def _build(loop_n=None, interleave=None, tiny_exp=False, no_recip=False,
           no_mask=False):
    """tiny_exp/no_recip/no_mask are timing-only diagnostics that cripple
    correctness."""
    if interleave is None:
        interleave = INTERLEAVE
    nc = bacc.Bacc("TRN2", target_bir_lowering=False, debug=False, num_devices=8)
    xT = nc.dram_tensor("xt", [D, T], BF16, kind="ExternalInput").ap()
    wq = nc.dram_tensor("wq", [D, DG], BF16, kind="ExternalInput").ap()
    wk = nc.dram_tensor("wk", [D, DG], BF16, kind="ExternalInput").ap()
    wv = nc.dram_tensor("wv", [D, DG], BF16, kind="ExternalInput").ap()
    wo = nc.dram_tensor("wo", [DG, D], BF16, kind="ExternalInput").ap()
    tri = nc.dram_tensor("tri", [P, P], BF16, kind="ExternalInput").ap()
    ones = nc.dram_tensor("ones", [P, P], BF16, kind="ExternalInput").ap()
    # esel[:, 128p:128(p+1)] selects/broadcasts recip-tile row 32*((2p+hi)%4)
    # into output partition half hi (norm's selector/broadcast matmul)
    esel = nc.dram_tensor("esel", [P, NPAIR * P], F32R,
                          kind="ExternalInput").ap()
    out = nc.dram_tensor("out", [T, D], F32, kind="ExternalOutput").ap()

    xT_r = xT.rearrange("(dc p) q -> p dc q", p=P)
    wq_r = wq.rearrange("(dc p) n -> p dc n", p=P)
    wk_r = wk.rearrange("(dc p) n -> p dc n", p=P)
    wv_r = wv.rearrange("(dc p) n -> p dc n", p=P)
    wo_r = wo.rearrange("(c p) n -> p c n", p=P)
    out_r = out.rearrange("(qc p) n -> qc p n", p=P)

    with tile.TileContext(nc) as tc:
        with ExitStack() as top:
            pers = top.enter_context(tc.tile_pool(name="persist", bufs=1))
            xsb = pers.tile([P, NDC, T], BF16)
            qt_sb = pers.tile([P, NPAIR, T], BF16)
            kt_sb = pers.tile([P, NPAIR, T], BF16)
            v_sb = pers.tile([P, NKC, GH, HDIM + 1], BF16)
            ctxT = pers.tile([P, NPAIR, T], BF16)
            wq_sb = pers.tile([P, NDC, DG], BF16)
            wk_sb = pers.tile([P, NDC, DG], BF16)
            wv_sb = pers.tile([P, NDC, DG], BF16)
            wo_sb = pers.tile([P, NPAIR, D], BF16)
            tri_sb = pers.tile([P, P], BF16)
            ones_sb = pers.tile([P, P], BF16)
            esel_sb = pers.tile([P, NPAIR * P], F32R)
            warm = pers.tile([1, 2], BF16)

            # x + small consts on the SP HWDGE ring; weights on the ACT ring
            for xi in range(NQB):
                nc.sync.dma_start(
                    xsb[:, :, xi * QB:(xi + 1) * QB],
                    xT_r[:, :, xi * QB:(xi + 1) * QB],
                )
            nc.sync.dma_start(tri_sb[:], tri)
            nc.sync.dma_start(ones_sb[:], ones)
            nc.sync.dma_start(esel_sb[:], esel)
            nc.scalar.dma_start(wq_sb[:], wq_r)
            nc.scalar.dma_start(wk_sb[:], wk_r)
            nc.scalar.dma_start(wv_sb[:], wv_r)
            nc.scalar.dma_start(wo_sb[:], wo_r)
            # preload the exp table while DMAs stream
            with nc.allow_low_precision(reason="table warmup"):
                nc.scalar.activation(warm[0:1, :], ones_sb[0:1, 0:2], AF.Exp)
            # ones-column of V (denominator trick)
            with nc.allow_low_precision(reason="bf16 store"):
                nc.vector.tensor_copy(
                    v_sb[:, :, :, HDIM],
                    ones_sb.rearrange("p (a b) -> p a b", a=NKC, b=GH),
                )

            body = ExitStack()
            st_psp = body.enter_context(
                tc.tile_pool(name="st_ps", bufs=ST_BUFS, space="PSUM"))
            ctx_psp = body.enter_context(
                tc.tile_pool(name="ctx_ps", bufs=2, space="PSUM"))
            genp = body.enter_context(
                tc.tile_pool(name="gen_ps", bufs=2, space="PSUM"))
            ptp = body.enter_context(tc.tile_pool(name="ptp", bufs=PT_BUFS))
            rcp = body.enter_context(tc.tile_pool(name="rcp", bufs=2))
            cup = body.enter_context(tc.tile_pool(name="cup", bufs=10))
            otp = body.enter_context(tc.tile_pool(name="otp", bufs=3))

            # ---- work units ----
            def qk_unit(w_sb, dst, pair, xi):
                qcols = slice(xi * QB, (xi + 1) * QB)
                pps = genp.tile([P, QB], F32, name="gen", tag="gen")
                for dc in range(NDC):
                    nc.tensor.matmul(
                        pps[:],
                        w_sb[:, dc, pair * P:(pair + 1) * P],
                        xsb[:, dc, qcols],
                        start=(dc == 0), stop=(dc == NDC - 1),
                    )
                with nc.allow_low_precision(reason="bf16 store"):
                    nc.vector.tensor_copy(dst[:, pair, qcols], pps[:])

            def v_unit(kc):
                vps = genp.tile([P, DG], F32, name="gen", tag="gen")
                for dc in range(NDC):
                    nc.tensor.matmul(
                        vps[:],
                        xsb[:, dc, kc * P:(kc + 1) * P],
                        wv_sb[:, dc, :],
                        start=(dc == 0), stop=(dc == NDC - 1),
                    )
                with nc.allow_low_precision(reason="bf16 store"):
                    nc.vector.tensor_copy(
                        v_sb[:, kc, :, 0:HDIM],
                        vps.rearrange("p (h d) -> p h d", d=HDIM),
                    )

            def att_iter(qb, pair, kc, nkc, ctxp):
                r = P * kc - QB * qb
                lo = max(r, 0)
                st = st_psp.tile([P, 2, QB], F32, name="stps", tag="st")
                pt = ptp.tile([P, 2, QB], BF16, name="pt")
                for hi in range(2):
                    nc.tensor.matmul(
                        st[:, hi, lo:QB],
                        kt_sb[HDIM * hi:HDIM * (hi + 1), pair,
                              kc * P:(kc + 1) * P],
                        qt_sb[HDIM * hi:HDIM * (hi + 1), pair,
                              qb * QB + lo:(qb + 1) * QB],
                        start=True, stop=True,
                    )
                ehi = lo + 2 if tiny_exp else QB
                with nc.allow_low_precision(reason="bf16 probs"):
                    nc.scalar.activation(
                        pt[:, :, lo:ehi], st[:, :, lo:ehi], AF.Exp,
                        scale=float(SCALE))
                if r >= 0 and not no_mask:
                    for hi in range(2):
                        with nc.allow_low_precision(reason="bf16 probs"):
                            nc.vector.tensor_tensor(
                                pt[:, hi, r:r + P],
                                pt[:, hi, r:r + P],
                                tri_sb[:],
                                mybir.AluOpType.mult,
                            )
                for hi in range(2):
                    nc.tensor.matmul(
                        ctxp[hi][:, lo:QB],
                        v_sb[:, kc, 2 * pair + hi, :],
                        pt[:, hi, lo:QB],
                        start=(kc == 0), stop=(kc == nkc - 1),
                    )

            def evac(qb, pair, ctxp, dens, ctxus):
                # evacuate ctx to SBUF (releases PSUM fast) and gather the
                # denominator row into a den tile for the per-qb batched
                # reciprocal.  Engine APs may only start at partition
                # 0/32/64/96, so j=2*pair+hi maps to tile j//4, row 32*(j%4).
                for hi in range(2):
                    ctxu = cup.tile([HDIM, QB], BF16, name="ctxu")
                    with nc.allow_low_precision(reason="bf16 ctx"):
                        nc.vector.tensor_copy(ctxu[:], ctxp[hi][0:HDIM, :])
                    j = 2 * pair + hi
                    ctxus[j] = ctxu
                    r = 32 * (j % 4)
                    nc.vector.tensor_copy(
                        dens[j // 4][r:r + 1, :],
                        ctxp[hi][HDIM:HDIM + 1, :])

            def norm(qb, dens, ctxus):
                # batched reciprocal: DVE reciprocal is iterative
                # (~8 cyc/elem on HW) and free-size-bound, so one call per
                # den tile (4 denominators on partitions 0/32/64/96) is ~4x
                # cheaper than per-head [1, QB] calls.  Unused partitions
                # hold garbage whose reciprocal is never read.  Then one
                # K=128 selector matmul per pair broadcasts both heads'
                # recip rows into [128, QB] PSUM and the normalize multiply
                # reads it straight from there.
                qcols = slice(qb * QB, (qb + 1) * QB)
                recips = []
                for t in range(2):
                    rc = rcp.tile([P, QB], F32R, name=f"recip{t}",
                                  tag=f"recip{t}")
                    with nc.allow_low_precision(reason="fp32r recip"):
                        if no_recip:
                            nc.vector.tensor_copy(rc[:], dens[t][:])
                        else:
                            nc.vector.reciprocal(rc[:], dens[t][:])
                    recips.append(rc)
                for pair in range(NPAIR):
                    bc = genp.tile([P, QB], F32, name="gen", tag="gen")
                    nc.tensor.matmul(
                        bc[:], esel_sb[:, P * pair:P * (pair + 1)],
                        recips[pair // 2][:, :], start=True, stop=True)
                    for hi in range(2):
                        j = 2 * pair + hi
                        with nc.allow_low_precision(reason="bf16 ctx"):
                            nc.vector.tensor_tensor(
                                ctxT[HDIM * hi:HDIM * (hi + 1), pair, qcols],
                                ctxus[j][:],
                                bc[HDIM * hi:HDIM * (hi + 1), :],
                                mybir.AluOpType.mult,
                            )

            def out_unit(qc):
                ot = otp.tile([P, D], F32, name="ot")
                for ob in range(2):
                    ops = genp.tile([P, 512], F32, name="gen", tag="gen")
                    for c in range(NPAIR):
                        nc.tensor.matmul(
                            ops[:],
                            ctxT[:, c, qc * P:(qc + 1) * P],
                            wo_sb[:, c, ob * 512:(ob + 1) * 512],
                            start=(c == 0), stop=(c == NPAIR - 1),
                        )
                    nc.vector.tensor_copy(ot[:, ob * 512:(ob + 1) * 512],
                                          ops[:])
                nc.sync.dma_start(out_r[qc], ot[:])

            def proj_round(xi):
                units = []
                for w_sb, dst in ((wq_sb, qt_sb), (wk_sb, kt_sb)):
                    for pair in range(NPAIR):
                        units.append(lambda w=w_sb, d=dst, p=pair, x=xi:
                                     qk_unit(w, d, p, x))
                for kl in range(QB // P):
                    units.append(lambda k=xi * (QB // P) + kl: v_unit(k))
                return units

            def out_round(qb):
                return [lambda q=4 * qb + j: out_unit(q) for j in range(4)]

            # ---- emission ----
            lp = (tc.For_i(0, loop_n, 1, hint_engines=(mybir.EngineType.PE,))
                  if loop_n else nullcontext())
            with lp:
                if interleave:
                    for u in proj_round(0):
                        u()
                    for qb in range(NQB):
                        fillers = []
                        if qb + 1 < NQB:
                            fillers += proj_round(qb + 1)
                        if qb - 1 >= 0:
                            fillers += out_round(qb - 1)
                        nkc = (QB // P) * (qb + 1)
                        n_att = NPAIR * nkc
                        dens = [rcp.tile([P, QB], F32, name=f"den{t}",
                                         tag=f"den{t}") for t in range(2)]
                        for d in dens:
                            # unused partitions must be finite: the selector
                            # matmul reads all 128 rows (0 * inf = NaN)
                            nc.gpsimd.memset(d[:], 1.0)
                        ctxus = [None] * (2 * NPAIR)
                        # spread fillers evenly among attention iterations
                        fi = 0
                        ai = 0
                        for pair in range(NPAIR):
                            ctxp = [
                                ctx_psp.tile([HDIM + 1, QB], F32,
                                             name="ctxps", tag="ctx")
                                for _ in range(2)
                            ]
                            for kc in range(nkc):
                                att_iter(qb, pair, kc, nkc, ctxp)
                                ai += 1
                                while fi * n_att < ai * len(fillers):
                                    fillers[fi]()
                                    fi += 1
                            evac(qb, pair, ctxp, dens, ctxus)
                        while fi < len(fillers):
                            fillers[fi]()
                            fi += 1
                        norm(qb, dens, ctxus)
                    for u in out_round(NQB - 1):
                        u()
                else:
                    for xi in range(NQB):
                        for u in proj_round(xi):
                            u()
                    for qb in range(NQB):
                        nkc = (QB // P) * (qb + 1)
                        dens = [rcp.tile([P, QB], F32, name=f"den{t}",
                                         tag=f"den{t}") for t in range(2)]
                        for d in dens:
                            # unused partitions must be finite: the selector
                            # matmul reads all 128 rows (0 * inf = NaN)
                            nc.gpsimd.memset(d[:], 1.0)
                        ctxus = [None] * (2 * NPAIR)
                        for pair in range(NPAIR):
                            ctxp = [
                                ctx_psp.tile([HDIM + 1, QB], F32,
                                             name="ctxps", tag="ctx")
                                for _ in range(2)
                            ]
                            for kc in range(nkc):
                                att_iter(qb, pair, kc, nkc, ctxp)
                            evac(qb, pair, ctxp, dens, ctxus)
                        norm(qb, dens, ctxus)
                    for qb in range(NQB):
                        for u in out_round(qb):
                            u()
            body.close()

    nc.compile()
    return nc


def _get_nc():
    if "nc" not in _CACHE:
        _CACHE["nc"] = _build()
    return _CACHE["nc"]


def make_in_maps(inputs, W_q, W_k, W_v, W_o):
    bf16 = mybir.dt.np(BF16)
    x = np.asarray(inputs, dtype=np.float32)
    W_q = np.asarray(W_q, dtype=np.float32)
    W_k = np.asarray(W_k, dtype=np.float32)
    W_v = np.asarray(W_v, dtype=np.float32)
    W_o = np.asarray(W_o, dtype=np.float32)
    tri = np.where(
        np.arange(P)[:, None] <= np.arange(P)[None, :], 1.0, 0.0
    ).astype(bf16)
    ones = np.ones((P, P), dtype=bf16)
    k_idx = np.arange(P)[:, None]
    m_idx = np.arange(NPAIR * P)[None, :]
    esel = (k_idx == 32 * ((2 * (m_idx // P) + (m_idx % P) // HDIM) % 4)
            ).astype(np.float32)
    in_maps = []
    for c in range(8):
        b, g = divmod(c, 2)
        gs = slice(g * DG, (g + 1) * DG)
        in_maps.append({
            "xt": np.ascontiguousarray(x[b].T).astype(bf16),
            "wq": np.ascontiguousarray(W_q[gs, :].T).astype(bf16),
            "wk": np.ascontiguousarray(W_k[gs, :].T).astype(bf16),
            "wv": np.ascontiguousarray(W_v[gs, :].T).astype(bf16),
            "wo": np.ascontiguousarray(W_o[:, gs].T).astype(bf16),
            "tri": tri,
            "ones": ones,
            "esel": esel,
        })
    return in_maps


def combine(results, b_o):
    b_o = np.asarray(b_o, dtype=np.float32)
    out = np.empty((B, T, D), dtype=np.float32)
    for b in range(B):
        out[b] = results[2 * b]["out"] + results[2 * b + 1]["out"] + b_o
    return out


def kernel(inputs, W_q, W_k, W_v, W_o, b_o):
    nc = _get_nc()
    in_maps = make_in_maps(inputs, W_q, W_k, W_v, W_o)
    res = run_bass_kernel_spmd(nc, in_maps, core_ids=list(range(8)), trace=False)
    return combine(res.results, b_o)
